# revision 4
# baseline (speedup 1.0000x reference)
"""Bass/Trainium2 kernel for nn_DDSOpWithReductionOpModel.

Computes out = nonzero(x).sum(dim=0) for x [8192, 8192] fp32 -> [2] int64.

Data-parallel over 8 NeuronCores, rows sharded 1024/core. Per core:
  - stream 2MB sub-tiles [128, 4096] fp32 from HBM on the two HWDGE rings
    (sync/scalar alternating), column-major piece order so banks 0-2 read
    out mid-stream and only bank 3 is on the tail; the last piece is
    split 2048/1024/1024 to keep the final dependency chain short.
    4096-col pieces keep the 8-lane DMA-completion-recycle window (~39us)
    well above the worst observed per-piece straggler lag (~17us), so the
    issue side never stalls even when one SDMA engine runs slow
  - DVE: per piece, mask = (x != 0) bf16 via tensor_scalar on [0, w-1)
    plus a single-column op (the split keeps the op shapes identical to
    the measured-good configuration)
  - PE: per 512-col chunk, stationary [128, 32] bf16 =
    [ones | iota_lo | iota_hi | zeros...] per row tile; PSUM accumulates
    across the 8 row tiles: rows 32g+{0,1,2} = {count, lo-sum, hi-sum}
  - readout: banks 0-2 whole-bank via gpsimd (SWDGE) mid-stream; bank 3
    via 3 tiny strided row-DMAs on sync right at the end
  Host combines: local row sums + core_off * count; all integer-exact.

Note on timing variance: the device is bimodal (~90 vs ~108 us for the
pure DMA stream) depending on sibling-NC HBM activity; this kernel
minimizes the invariant overhead (ramp + tail) on top of that floor.
"""

import ml_dtypes
import numpy as np

import concourse.bacc as bacc
import concourse.mybir as mybir
from concourse.bass_utils import run_bass_kernel_spmd
from concourse.tile import TileContext

N0, N1 = 8192, 8192
N_CORES = 8
R = N0 // N_CORES
CHUNK = 512
SW = 32


def tile_plan(rows=R, cols=N1, sub_cols=4096, tail_split=True):
    nt = rows // 128
    plan = []
    for s in range(cols // sub_cols):
        for t in range(nt):
            plan.append((t, s * sub_cols, sub_cols))
    if tail_split and sub_cols % (4 * CHUNK) == 0:
        t, c0, w = plan.pop()
        plan += [(t, c0, w // 2), (t, c0 + w // 2, w // 4), (t, c0 + 3 * w // 4, w // 4)]
    return plan


def make_wtab(rows=R):
    nt = rows // 128
    w = np.zeros((128, SW * nt), dtype=np.float32)
    p = np.arange(128)
    for t in range(nt):
        local = t * 128 + p
        w[:, SW * t + 0] = 1.0
        w[:, SW * t + 1] = local % 64
        w[:, SW * t + 2] = local // 64
    return w.astype(ml_dtypes.bfloat16)


def build_nc(
    rows=R,
    cols=N1,
    sub_cols=4096,
    tail_split=True,
    x_bufs=7,
    mask_bufs=5,
    dual_dma=True,
):
    assert rows % 128 == 0 and cols % CHUNK == 0 and sub_cols % CHUNK == 0
    plan = tile_plan(rows, cols, sub_cols, tail_split)
    nt = rows // 128
    n_chunks = cols // CHUNK
    n_banks = (n_chunks + 3) // 4
    assert n_banks <= 8

    touches = []
    for i, (t, c0, w) in enumerate(plan):
        for j in range(w // CHUNK):
            ch = (c0 + j * CHUNK) // CHUNK
            touches.append((i, j, ch, ch // 4))
    last_touch = {}
    chunk_first = {}
    chunk_last = {}
    for i, j, ch, b in touches:
        last_touch[b] = (i, j)
        chunk_first.setdefault(ch, (i, j))
        chunk_last[ch] = (i, j)
    final_bank = max(last_touch, key=lambda b: last_touch[b])

    nc = bacc.Bacc("TRN2", target_bir_lowering=False)
    x = nc.dram_tensor("x", [rows, cols], mybir.dt.float32, kind="ExternalInput")
    w_in = nc.dram_tensor(
        "w", [128, SW * nt], mybir.dt.bfloat16, kind="ExternalInput"
    )
    col_out = nc.dram_tensor(
        "col_out", [n_banks * 128, CHUNK], mybir.dt.float32, kind="ExternalOutput"
    )

    with TileContext(nc) as tc:
        with (
            tc.tile_pool(name="xp", bufs=x_bufs) as xp,
            tc.tile_pool(name="mp", bufs=mask_bufs) as mp,
            tc.tile_pool(name="pp", bufs=1, space="PSUM") as pp,
            tc.tile_pool(name="cp", bufs=1) as cp,
        ):
            wt = cp.tile([128, SW * nt], mybir.dt.bfloat16)
            nc.gpsimd.dma_start(out=wt, in_=w_in.ap())
            psums = [
                pp.tile([128, CHUNK], mybir.dt.float32, name=f"psum{b}")
                for b in range(n_banks)
            ]
            col_sbs = [
                cp.tile([128, CHUNK], mybir.dt.float32, name=f"colsb{b}")
                for b in range(n_banks)
            ]
            for i, (t, c0, w) in enumerate(plan):
                xt = xp.tile([128, w], mybir.dt.float32, name=f"xt{i}", tag="x")
                if dual_dma and i % 2 == 1:
                    dma_eng = nc.scalar
                else:
                    dma_eng = nc.sync
                dma_eng.dma_start(
                    out=xt, in_=x[t * 128 : (t + 1) * 128, c0 : c0 + w]
                )
                mt = mp.tile([128, w], mybir.dt.bfloat16, name=f"mt{i}", tag="m")
                nc.vector.tensor_scalar(
                    out=mt[:, 0 : w - 1],
                    in0=xt[:, 0 : w - 1],
                    scalar1=0.0,
                    scalar2=None,
                    op0=mybir.AluOpType.not_equal,
                )
                nc.vector.tensor_scalar(
                    out=mt[:, w - 1 : w],
                    in0=xt[:, w - 1 : w],
                    scalar1=0.0,
                    scalar2=None,
                    op0=mybir.AluOpType.not_equal,
                )
                for j in range(w // CHUNK):
                    ch = (c0 + j * CHUNK) // CHUNK
                    b, g = ch // 4, ch % 4
                    nc.tensor.matmul(
                        psums[b][32 * g : 32 * g + 32, :],
                        lhsT=wt[:, SW * t : SW * (t + 1)],
                        rhs=mt[:, j * CHUNK : (j + 1) * CHUNK],
                        start=(chunk_first[ch] == (i, j)),
                        stop=(chunk_last[ch] == (i, j)),
                        tile_position=(0, 32 * g),
                        skip_group_check=True,
                    )
                    if last_touch[b] == (i, j):
                        # the two final banks' copies run on different
                        # engines so they overlap; the last bank reads out
                        # on the (idle by now) sync ring
                        if b == final_bank:
                            nc.vector.tensor_copy(out=col_sbs[b], in_=psums[b])
                            nc.sync.dma_start(
                                out=col_out[b * 128 : (b + 1) * 128, :],
                                in_=col_sbs[b],
                            )
                        elif b == final_bank - 1:
                            nc.scalar.copy(out=col_sbs[b], in_=psums[b])
                            nc.gpsimd.dma_start(
                                out=col_out[b * 128 : (b + 1) * 128, :],
                                in_=col_sbs[b],
                            )
                        else:
                            nc.vector.tensor_copy(out=col_sbs[b], in_=psums[b])
                            nc.gpsimd.dma_start(
                                out=col_out[b * 128 : (b + 1) * 128, :],
                                in_=col_sbs[b],
                            )
    nc.compile()
    return nc


_NC_CACHE = {}


def _get_nc():
    if "nc" not in _NC_CACHE:
        _NC_CACHE["nc"] = build_nc()
    return _NC_CACHE["nc"]


def postprocess(results, rows=R, cols=N1):
    n_chunks = cols // CHUNK
    n_banks = (n_chunks + 3) // 4
    out_rows = np.int64(0)
    col_counts = np.zeros(cols, dtype=np.int64)
    for core, res in enumerate(results):
        co = np.rint(np.asarray(res["col_out"], dtype=np.float64)).astype(np.int64)
        co = co.reshape(n_banks, 128, CHUNK)
        counts = np.zeros((n_chunks, CHUNK), dtype=np.int64)
        lo = np.zeros((n_chunks, CHUNK), dtype=np.int64)
        hi = np.zeros((n_chunks, CHUNK), dtype=np.int64)
        for ch in range(n_chunks):
            b, g = ch // 4, ch % 4
            counts[ch] = co[b, 32 * g + 0]
            lo[ch] = co[b, 32 * g + 1]
            hi[ch] = co[b, 32 * g + 2]
        local_rowsum = lo.sum() + 64 * hi.sum()
        out_rows += local_rowsum + np.int64(core) * rows * counts.sum()
        col_counts += counts.reshape(cols)
    out_cols = np.dot(np.arange(cols, dtype=np.int64), col_counts)
    return np.array([out_rows, out_cols], dtype=np.int64)


def kernel(inputs, _trace=False, _trace_kwargs=None):
    x = np.ascontiguousarray(np.asarray(inputs, dtype=np.float32))
    assert x.shape == (N0, N1)
    wtab = make_wtab()
    in_maps = [
        {"x": x[c * R : (c + 1) * R], "w": wtab} for c in range(N_CORES)
    ]
    res = run_bass_kernel_spmd(
        _get_nc(),
        in_maps,
        core_ids=list(range(N_CORES)),
        trace=_trace,
        **(_trace_kwargs or {}),
    )
    out = postprocess(res.results)
    if _trace:
        return out, res
    return out


# revision 5
# speedup vs baseline: 1.1501x; 1.1501x over previous
"""Bass/Trainium2 kernel for nn_DDSOpWithReductionOpModel.

Computes out = nonzero(x).sum(dim=0) for x [8192, 8192] fp32 -> [2] int64.

Data-parallel over 8 NeuronCores, rows sharded 1024/core. Per core:
  - stream 2MB sub-tiles [128, 4096] fp32 from HBM on the two HWDGE rings
    (sync/scalar alternating), column-major piece order so banks 0-2 read
    out mid-stream and only bank 3 is on the tail; the last piece is
    split 2048/1024/1024 to keep the final dependency chain short.
    4096-col pieces keep the 8-lane DMA-completion-recycle window (~39us)
    well above the worst observed per-piece straggler lag (~17us), so the
    issue side never stalls even when one SDMA engine runs slow
  - DVE: per piece, mask = (x != 0) bf16 via tensor_scalar on [0, w-1)
    plus a single-column op (the split keeps the op shapes identical to
    the measured-good configuration)
  - PE: per 512-col chunk, stationary [128, 32] bf16 =
    [ones | iota_lo | iota_hi | zeros...] per row tile; PSUM accumulates
    across the 8 row tiles: rows 32g+{0,1,2} = {count, lo-sum, hi-sum}
  - readout: banks 0-2 whole-bank via gpsimd (SWDGE) mid-stream; bank 3
    via 3 tiny strided row-DMAs on sync right at the end
  Host combines: local row sums + core_off * count; all integer-exact.

Note on timing variance: the device is bimodal (~90 vs ~108 us for the
pure DMA stream) depending on sibling-NC HBM activity; this kernel
minimizes the invariant overhead (ramp + tail) on top of that floor.
"""

import ml_dtypes
import numpy as np

import concourse.bacc as bacc
import concourse.mybir as mybir
from concourse.bass_utils import run_bass_kernel_spmd
from concourse.tile import TileContext

N0, N1 = 8192, 8192
N_CORES = 8
R = N0 // N_CORES
CHUNK = 512
SW = 32


def tile_plan(rows=R, cols=N1, sub_cols=4096, tail_split=True):
    nt = rows // 128
    plan = []
    for s in range(cols // sub_cols):
        for t in range(nt):
            plan.append((t, s * sub_cols, sub_cols))
    if tail_split and sub_cols % (8 * CHUNK) == 0:
        t, c0, w = plan.pop()
        plan += [
            (t, c0, w // 2),
            (t, c0 + w // 2, w // 4),
            (t, c0 + 3 * w // 4, w // 8),
            (t, c0 + 7 * w // 8, w // 8),
        ]
    return plan


def make_wtab(rows=R):
    nt = rows // 128
    w = np.zeros((128, SW * nt), dtype=np.float32)
    p = np.arange(128)
    for t in range(nt):
        local = t * 128 + p
        w[:, SW * t + 0] = 1.0
        w[:, SW * t + 1] = local % 64
        w[:, SW * t + 2] = local // 64
    return w.astype(ml_dtypes.bfloat16)


def build_nc(
    rows=R,
    cols=N1,
    sub_cols=4096,
    tail_split=True,
    x_bufs=7,
    mask_bufs=5,
    dual_dma=True,
):
    assert rows % 128 == 0 and cols % CHUNK == 0 and sub_cols % CHUNK == 0
    plan = tile_plan(rows, cols, sub_cols, tail_split)
    nt = rows // 128
    n_chunks = cols // CHUNK
    n_banks = (n_chunks + 3) // 4
    assert n_banks <= 8

    touches = []
    for i, (t, c0, w) in enumerate(plan):
        for j in range(w // CHUNK):
            ch = (c0 + j * CHUNK) // CHUNK
            touches.append((i, j, ch, ch // 4))
    last_touch = {}
    chunk_first = {}
    chunk_last = {}
    for i, j, ch, b in touches:
        last_touch[b] = (i, j)
        chunk_first.setdefault(ch, (i, j))
        chunk_last[ch] = (i, j)
    final_bank = max(last_touch, key=lambda b: last_touch[b])

    nc = bacc.Bacc("TRN2", target_bir_lowering=False)
    x = nc.dram_tensor("x", [rows, cols], mybir.dt.float32, kind="ExternalInput")
    w_in = nc.dram_tensor(
        "w", [128, SW * nt], mybir.dt.bfloat16, kind="ExternalInput"
    )
    col_out = nc.dram_tensor(
        "col_out", [n_banks * 128, CHUNK], mybir.dt.float32, kind="ExternalOutput"
    )

    with TileContext(nc) as tc:
        with (
            tc.tile_pool(name="xp", bufs=x_bufs) as xp,
            tc.tile_pool(name="mp", bufs=mask_bufs) as mp,
            tc.tile_pool(name="pp", bufs=1, space="PSUM") as pp,
            tc.tile_pool(name="cp", bufs=1) as cp,
        ):
            wt = cp.tile([128, SW * nt], mybir.dt.bfloat16)
            nc.gpsimd.dma_start(out=wt, in_=w_in.ap())
            psums = [
                pp.tile([128, CHUNK], mybir.dt.float32, name=f"psum{b}")
                for b in range(n_banks)
            ]
            col_sbs = [
                cp.tile([128, CHUNK], mybir.dt.float32, name=f"colsb{b}")
                for b in range(n_banks)
            ]
            for i, (t, c0, w) in enumerate(plan):
                xt = xp.tile([128, w], mybir.dt.float32, name=f"xt{i}", tag="x")
                if dual_dma and i % 2 == 1:
                    dma_eng = nc.scalar
                else:
                    dma_eng = nc.sync
                dma_eng.dma_start(
                    out=xt, in_=x[t * 128 : (t + 1) * 128, c0 : c0 + w]
                )
                mt = mp.tile([128, w], mybir.dt.bfloat16, name=f"mt{i}", tag="m")
                nc.vector.tensor_scalar(
                    out=mt[:, 0 : w - 1],
                    in0=xt[:, 0 : w - 1],
                    scalar1=0.0,
                    scalar2=None,
                    op0=mybir.AluOpType.not_equal,
                )
                nc.vector.tensor_scalar(
                    out=mt[:, w - 1 : w],
                    in0=xt[:, w - 1 : w],
                    scalar1=0.0,
                    scalar2=None,
                    op0=mybir.AluOpType.not_equal,
                )
                for j in range(w // CHUNK):
                    ch = (c0 + j * CHUNK) // CHUNK
                    b, g = ch // 4, ch % 4
                    nc.tensor.matmul(
                        psums[b][32 * g : 32 * g + 32, :],
                        lhsT=wt[:, SW * t : SW * (t + 1)],
                        rhs=mt[:, j * CHUNK : (j + 1) * CHUNK],
                        start=(chunk_first[ch] == (i, j)),
                        stop=(chunk_last[ch] == (i, j)),
                        tile_position=(0, 32 * g),
                        skip_group_check=True,
                    )
                    if last_touch[b] == (i, j):
                        # the two final banks' copies run on different
                        # engines so they overlap; the last bank reads out
                        # on the (idle by now) sync ring
                        if b == final_bank:
                            nc.scalar.copy(out=col_sbs[b], in_=psums[b])
                            nc.sync.dma_start(
                                out=col_out[b * 128 : (b + 1) * 128, :],
                                in_=col_sbs[b],
                            )
                        elif b == final_bank - 1:
                            nc.vector.tensor_copy(out=col_sbs[b], in_=psums[b])
                            nc.gpsimd.dma_start(
                                out=col_out[b * 128 : (b + 1) * 128, :],
                                in_=col_sbs[b],
                            )
                        else:
                            nc.vector.tensor_copy(out=col_sbs[b], in_=psums[b])
                            nc.gpsimd.dma_start(
                                out=col_out[b * 128 : (b + 1) * 128, :],
                                in_=col_sbs[b],
                            )
    nc.compile()
    return nc


_NC_CACHE = {}


def _get_nc():
    if "nc" not in _NC_CACHE:
        _NC_CACHE["nc"] = build_nc()
    return _NC_CACHE["nc"]


def postprocess(results, rows=R, cols=N1):
    n_chunks = cols // CHUNK
    n_banks = (n_chunks + 3) // 4
    out_rows = np.int64(0)
    col_counts = np.zeros(cols, dtype=np.int64)
    for core, res in enumerate(results):
        co = np.rint(np.asarray(res["col_out"], dtype=np.float64)).astype(np.int64)
        co = co.reshape(n_banks, 128, CHUNK)
        counts = np.zeros((n_chunks, CHUNK), dtype=np.int64)
        lo = np.zeros((n_chunks, CHUNK), dtype=np.int64)
        hi = np.zeros((n_chunks, CHUNK), dtype=np.int64)
        for ch in range(n_chunks):
            b, g = ch // 4, ch % 4
            counts[ch] = co[b, 32 * g + 0]
            lo[ch] = co[b, 32 * g + 1]
            hi[ch] = co[b, 32 * g + 2]
        local_rowsum = lo.sum() + 64 * hi.sum()
        out_rows += local_rowsum + np.int64(core) * rows * counts.sum()
        col_counts += counts.reshape(cols)
    out_cols = np.dot(np.arange(cols, dtype=np.int64), col_counts)
    return np.array([out_rows, out_cols], dtype=np.int64)


def kernel(inputs, _trace=False, _trace_kwargs=None):
    x = np.ascontiguousarray(np.asarray(inputs, dtype=np.float32))
    assert x.shape == (N0, N1)
    wtab = make_wtab()
    in_maps = [
        {"x": x[c * R : (c + 1) * R], "w": wtab} for c in range(N_CORES)
    ]
    res = run_bass_kernel_spmd(
        _get_nc(),
        in_maps,
        core_ids=list(range(N_CORES)),
        trace=_trace,
        **(_trace_kwargs or {}),
    )
    out = postprocess(res.results)
    if _trace:
        return out, res
    return out


# revision 6
# speedup vs baseline: 1.1659x; 1.0137x over previous
"""Bass/Trainium2 kernel for nn_DDSOpWithReductionOpModel.

Computes out = nonzero(x).sum(dim=0) for x [8192, 8192] fp32 -> [2] int64.

Data-parallel over 8 NeuronCores, rows sharded 1024/core. Per core:
  - stream 2MB sub-tiles [128, 4096] fp32 from HBM on the two HWDGE rings
    (sync/scalar alternating), column-major piece order so banks 0-2 read
    out mid-stream and only bank 3 is on the tail; the last piece is
    split 2048/1024/1024 to keep the final dependency chain short.
    4096-col pieces keep the 8-lane DMA-completion-recycle window (~39us)
    well above the worst observed per-piece straggler lag (~17us), so the
    issue side never stalls even when one SDMA engine runs slow
  - DVE: per piece, mask = (x != 0) bf16 via tensor_scalar on [0, w-1)
    plus a single-column op (the split keeps the op shapes identical to
    the measured-good configuration)
  - PE: per 512-col chunk, stationary [128, 32] bf16 =
    [ones | iota_lo | iota_hi | zeros...] per row tile; PSUM accumulates
    across the 8 row tiles: rows 32g+{0,1,2} = {count, lo-sum, hi-sum}
  - readout: banks 0-2 whole-bank via gpsimd (SWDGE) mid-stream; bank 3
    via 3 tiny strided row-DMAs on sync right at the end
  Host combines: local row sums + core_off * count; all integer-exact.

Note on timing variance: the device is bimodal (~90 vs ~108 us for the
pure DMA stream) depending on sibling-NC HBM activity; this kernel
minimizes the invariant overhead (ramp + tail) on top of that floor.
"""

import ml_dtypes
import numpy as np

import concourse.bacc as bacc
import concourse.mybir as mybir
from concourse.bass_utils import run_bass_kernel_spmd
from concourse.tile import TileContext

N0, N1 = 8192, 8192
N_CORES = 8
R = N0 // N_CORES
CHUNK = 512
SW = 32


def tile_plan(rows=R, cols=N1, sub_cols=4096, tail_split=True):
    nt = rows // 128
    plan = []
    for s in range(cols // sub_cols):
        for t in range(nt):
            plan.append((t, s * sub_cols, sub_cols))
    if tail_split and sub_cols % (8 * CHUNK) == 0:
        t, c0, w = plan.pop()
        plan += [
            (t, c0, w // 2),
            (t, c0 + w // 2, w // 4),
            (t, c0 + 3 * w // 4, w // 8),
            (t, c0 + 7 * w // 8, w // 8),
        ]
    return plan


def make_wtab(rows=R):
    nt = rows // 128
    w = np.zeros((128, SW * nt), dtype=np.float32)
    p = np.arange(128)
    for t in range(nt):
        local = t * 128 + p
        w[:, SW * t + 0] = 1.0
        w[:, SW * t + 1] = local % 64
        w[:, SW * t + 2] = local // 64
    return w.astype(ml_dtypes.bfloat16)


def build_nc(
    rows=R,
    cols=N1,
    sub_cols=4096,
    tail_split=True,
    x_bufs=7,
    mask_bufs=5,
    dual_dma=True,
):
    assert rows % 128 == 0 and cols % CHUNK == 0 and sub_cols % CHUNK == 0
    plan = tile_plan(rows, cols, sub_cols, tail_split)
    nt = rows // 128
    n_chunks = cols // CHUNK
    n_banks = (n_chunks + 3) // 4
    assert n_banks <= 8

    touches = []
    for i, (t, c0, w) in enumerate(plan):
        for j in range(w // CHUNK):
            ch = (c0 + j * CHUNK) // CHUNK
            touches.append((i, j, ch, ch // 4))
    last_touch = {}
    chunk_first = {}
    chunk_last = {}
    for i, j, ch, b in touches:
        last_touch[b] = (i, j)
        chunk_first.setdefault(ch, (i, j))
        chunk_last[ch] = (i, j)
    final_bank = max(last_touch, key=lambda b: last_touch[b])

    nc = bacc.Bacc("TRN2", target_bir_lowering=False)
    x = nc.dram_tensor("x", [rows, cols], mybir.dt.float32, kind="ExternalInput")
    w_in = nc.dram_tensor(
        "w", [128, SW * nt], mybir.dt.bfloat16, kind="ExternalInput"
    )
    col_out = nc.dram_tensor(
        "col_out", [n_banks * 128, CHUNK], mybir.dt.float32, kind="ExternalOutput"
    )

    with TileContext(nc) as tc:
        with (
            tc.tile_pool(name="xp", bufs=x_bufs) as xp,
            tc.tile_pool(name="mp", bufs=mask_bufs) as mp,
            tc.tile_pool(name="pp", bufs=1, space="PSUM") as pp,
            tc.tile_pool(name="cp", bufs=1) as cp,
        ):
            wt = cp.tile([128, SW * nt], mybir.dt.bfloat16)
            nc.gpsimd.dma_start(out=wt, in_=w_in.ap())
            psums = [
                pp.tile([128, CHUNK], mybir.dt.float32, name=f"psum{b}")
                for b in range(n_banks)
            ]
            col_sbs = [
                cp.tile([128, CHUNK], mybir.dt.float32, name=f"colsb{b}")
                for b in range(n_banks)
            ]
            for i, (t, c0, w) in enumerate(plan):
                xt = xp.tile([128, w], mybir.dt.float32, name=f"xt{i}", tag="x")
                if dual_dma and i % 2 == 1:
                    dma_eng = nc.scalar
                else:
                    dma_eng = nc.sync
                dma_eng.dma_start(
                    out=xt, in_=x[t * 128 : (t + 1) * 128, c0 : c0 + w]
                )
                mt = mp.tile([128, w], mybir.dt.bfloat16, name=f"mt{i}", tag="m")
                nc.vector.tensor_scalar(
                    out=mt[:, 0 : w - 1],
                    in0=xt[:, 0 : w - 1],
                    scalar1=0.0,
                    scalar2=None,
                    op0=mybir.AluOpType.not_equal,
                )
                nc.vector.tensor_scalar(
                    out=mt[:, w - 1 : w],
                    in0=xt[:, w - 1 : w],
                    scalar1=0.0,
                    scalar2=None,
                    op0=mybir.AluOpType.not_equal,
                )
                for j in range(w // CHUNK):
                    ch = (c0 + j * CHUNK) // CHUNK
                    b, g = ch // 4, ch % 4
                    nc.tensor.matmul(
                        psums[b][32 * g : 32 * g + 32, :],
                        lhsT=wt[:, SW * t : SW * (t + 1)],
                        rhs=mt[:, j * CHUNK : (j + 1) * CHUNK],
                        start=(chunk_first[ch] == (i, j)),
                        stop=(chunk_last[ch] == (i, j)),
                        tile_position=(0, 32 * g),
                        skip_group_check=True,
                    )
                    if last_touch[b] == (i, j):
                        # the two final banks' copies run on different
                        # engines so they overlap; the last bank reads out
                        # on the (idle by now) sync ring
                        if b == final_bank:
                            nc.scalar.copy(out=col_sbs[b], in_=psums[b])
                            nc.sync.dma_start(
                                out=col_out[b * 128 : (b + 1) * 128, :],
                                in_=col_sbs[b],
                            )
                        elif b == final_bank - 1:
                            # ACT copy here warms the activation table
                            # mid-stream so the final bank's ACT copy has
                            # no table-load in the tail
                            nc.scalar.copy(out=col_sbs[b], in_=psums[b])
                            nc.gpsimd.dma_start(
                                out=col_out[b * 128 : (b + 1) * 128, :],
                                in_=col_sbs[b],
                            )
                        else:
                            nc.vector.tensor_copy(out=col_sbs[b], in_=psums[b])
                            nc.gpsimd.dma_start(
                                out=col_out[b * 128 : (b + 1) * 128, :],
                                in_=col_sbs[b],
                            )
    nc.compile()
    return nc


_NC_CACHE = {}


def _get_nc():
    if "nc" not in _NC_CACHE:
        _NC_CACHE["nc"] = build_nc()
    return _NC_CACHE["nc"]


def postprocess(results, rows=R, cols=N1):
    n_chunks = cols // CHUNK
    n_banks = (n_chunks + 3) // 4
    out_rows = np.int64(0)
    col_counts = np.zeros(cols, dtype=np.int64)
    for core, res in enumerate(results):
        co = np.rint(np.asarray(res["col_out"], dtype=np.float64)).astype(np.int64)
        co = co.reshape(n_banks, 128, CHUNK)
        counts = np.zeros((n_chunks, CHUNK), dtype=np.int64)
        lo = np.zeros((n_chunks, CHUNK), dtype=np.int64)
        hi = np.zeros((n_chunks, CHUNK), dtype=np.int64)
        for ch in range(n_chunks):
            b, g = ch // 4, ch % 4
            counts[ch] = co[b, 32 * g + 0]
            lo[ch] = co[b, 32 * g + 1]
            hi[ch] = co[b, 32 * g + 2]
        local_rowsum = lo.sum() + 64 * hi.sum()
        out_rows += local_rowsum + np.int64(core) * rows * counts.sum()
        col_counts += counts.reshape(cols)
    out_cols = np.dot(np.arange(cols, dtype=np.int64), col_counts)
    return np.array([out_rows, out_cols], dtype=np.int64)


def kernel(inputs, _trace=False, _trace_kwargs=None):
    x = np.ascontiguousarray(np.asarray(inputs, dtype=np.float32))
    assert x.shape == (N0, N1)
    wtab = make_wtab()
    in_maps = [
        {"x": x[c * R : (c + 1) * R], "w": wtab} for c in range(N_CORES)
    ]
    res = run_bass_kernel_spmd(
        _get_nc(),
        in_maps,
        core_ids=list(range(N_CORES)),
        trace=_trace,
        **(_trace_kwargs or {}),
    )
    out = postprocess(res.results)
    if _trace:
        return out, res
    return out
